# revision 5
# baseline (speedup 1.0000x reference)
"""MatchBRNN Trainium2 kernel: 2-layer action-conditioned-attention + bidirectional
SRU, data-parallel over batch on 8 NeuronCores.

Per-core work (B_local=2, S=L=256, D=256, H=128, K=64):
  memory = x; out0 = x^T
  for layer in 0,1:
    xtT[(b,k), l] = (memory_b @ w1a_b)^T      (layer-invariant, computed once)
    ytT[(b,k), s] = (out_b @ w2a_b)^T + (b1a+b2a)
    t[(b,k), l] = tanh(xtT + ytT[:, s])  per s   -> bf16
    scoresT[l, (s,b)] = va^T-blockdiag @ t       (PE, M=128 per l-half)
    e = exp(scoresT); Z = ones^T e; pools = (mem^T e) * (1/Z)
    U_j = W_j^T [out; pools]  (4 slices; f32r matmuls)
    c = scan(f, (1-f) u0);  h = r*tanh(c) + (1-r)*hw   (sigmoid via tanh)
  output = concat(h_f, h_b) per layer-2
"""
import numpy as np
import concourse.bass as bass
import concourse.mybir as mybir
import concourse.tile as tile
from concourse.bass_utils import run_bass_kernel_spmd

AF = mybir.ActivationFunctionType
OP = mybir.AluOpType
F32 = mybir.dt.float32
F32R = mybir.dt.float32r
BF16 = mybir.dt.bfloat16
BF16_NP = mybir.dt.np(BF16)

B, S, D = 16, 256, 256
H, NL, A, K = 128, 2, 8, 64
NCORES = 8
B2 = B // NCORES  # 2 local batches per core


def _split_excess_waits(nc, max_waits=1):
    """walrus in this toolchain rejects >1 sem-wait per instruction; hoist
    extras onto same-engine NoOps inserted just before the instruction."""
    n = 0
    for f in nc.m.functions:
        for bb in f.blocks:
            out = []
            for inst in bb.instructions:
                si = inst.sync_info
                waits = list(si.on_wait) if si is not None and si.on_wait else []
                if len(waits) > max_waits:
                    keep, extra = waits[-max_waits:], waits[:-max_waits]
                    for w in extra:
                        n += 1
                        out.append(mybir.InstNoOp(
                            name=f"{inst.name}_ws{n}", engine=inst.engine,
                            ins=[], outs=[],
                            sync_info=mybir.SyncInfo(on_wait=[w], on_update=[])))
                    inst.sync_info = mybir.SyncInfo(
                        on_wait=keep, on_update=list(si.on_update or []))
                out.append(inst)
            bb.instructions = out
    return n


def _build(apply_mask: bool):
    nc = bass.Bass("TRN2")
    dram = nc.dram_tensor
    memT_d = dram("memT", [128, 1024], F32R, kind="ExternalInput")
    memr_d = dram("memr", [128, 1024], F32R, kind="ExternalInput")
    w1_d = dram("w1blk", [128, 512], F32R, kind="ExternalInput")
    w2_d = dram("w2blk", [128, 512], F32R, kind="ExternalInput")
    va_d = dram("vablk", [128, 2], BF16, kind="ExternalInput")
    yb_d = dram("ybias", [128, 1], F32, kind="ExternalInput")
    ws_d = dram("wsru", [128, 8192], F32R, kind="ExternalInput")
    bs_d = dram("bsru", [128, 8], F32, kind="ExternalInput")
    oc_d = dram("onescol", [128, 1], F32R, kind="ExternalInput")
    or_d = dram("onesrow", [1, 128], F32R, kind="ExternalInput")
    if apply_mask:
        mk_d = dram("maskmul", [128, 4], F32, kind="ExternalInput")
    outT_d = dram("outT", [B2, 2, 128, 256], F32, kind="ExternalOutput")

    with tile.TileContext(nc) as tc:
        with (
            nc.allow_low_precision(reason="bf16/f32r staging is intentional"),
            tc.tile_pool(name="const", bufs=1) as cp,
            tc.tile_pool(name="work", bufs=1) as wp,
            tc.tile_pool(name="blk", bufs=3) as bp,
            tc.tile_pool(name="sru", bufs=2) as sp,
            tc.tile_pool(name="ps", bufs=8, space="PSUM") as ps,
        ):
            _bank_n = [0]

            def bank():
                _bank_n[0] += 1
                return ps.tile([128, 512], F32, tag="bank",
                               name=f"bank{_bank_n[0]}")

            memT = cp.tile([128, 1024], F32R, tag="memT")
            memr = cp.tile([128, 1024], F32R, tag="memr")
            w1 = cp.tile([128, 512], F32R, tag="w1")
            w2 = cp.tile([128, 512], F32R, tag="w2")
            va = cp.tile([128, 2], BF16, tag="va")
            yb = cp.tile([128, 1], F32, tag="yb")
            wsru = cp.tile([128, 8192], F32R, tag="wsru")
            bsru = cp.tile([128, 8], F32, tag="bsru")
            onc = cp.tile([128, 1], F32R, tag="onc")
            onr = cp.tile([1, 128], F32R, tag="onr")
            for t, d in ((memT, memT_d), (memr, memr_d), (w1, w1_d), (w2, w2_d),
                         (va, va_d), (yb, yb_d), (bsru, bs_d), (onc, oc_d),
                         (onr, or_d)):
                nc.sync.dma_start(t[:], d[:])
            # split the big SRU weight load across queues
            for q in range(4):
                nc.sync.dma_start(wsru[:, q * 2048:(q + 1) * 2048],
                                  ws_d[:, q * 2048:(q + 1) * 2048])
            if apply_mask:
                mk = cp.tile([128, 4], F32, tag="mk")
                nc.sync.dma_start(mk[:], mk_d[:])

            h0 = [wp.tile([128, 512], F32R, tag=f"h0{d}", name=f"h0{d}")
                  for d in range(2)]
            h1 = [wp.tile([128, 512], F32, tag=f"h1{d}", name=f"h1{d}")
                  for d in range(2)]

            # xtT: layer-invariant. contract over (b, d-half) with block-diag w1
            xt_ps = bank()
            for cc in range(4):
                b, ci = cc // 2, cc % 2
                nc.tensor.matmul(
                    xt_ps[:, 0:256], w1[:, cc * 128:(cc + 1) * 128],
                    memT[:, ci * 512 + b * 256: ci * 512 + b * 256 + 256],
                    start=(cc == 0), stop=(cc == 3))
            xt16 = wp.tile([128, 256], BF16, tag="xt16")
            nc.scalar.copy(xt16[:], xt_ps[:, 0:256])

            for li in range(NL):
                # ---- ytT = (out @ w2a)^T + ybias ----
                yt_ps = bank()
                for cc in range(4):
                    b, ci = cc // 2, cc % 2
                    if li == 0:
                        rhs = memT[:, ci * 512 + b * 256: ci * 512 + b * 256 + 256]
                    else:
                        rhs = h0[ci][:, b * 256: b * 256 + 256]
                    nc.tensor.matmul(
                        yt_ps[:, 0:256], w2[:, cc * 128:(cc + 1) * 128], rhs,
                        start=(cc == 0), stop=(cc == 3))
                yt = wp.tile([128, 256], F32, tag="yt")
                nc.vector.tensor_scalar(yt[:], yt_ps[:, 0:256], yb[:], None, OP.add)

                # ---- t = tanh(xt + yt[s]); scoresT via PE reduce over (b,k) ----
                sc_ps = [bank(), bank()]
                for blk in range(32):
                    tp = bp.tile([128, 2048], BF16, tag="tpre")
                    tb = bp.tile([128, 2048], BF16, tag="tblk")
                    for j in range(8):
                        s = blk * 8 + j
                        nc.vector.tensor_scalar(
                            tp[:, j * 256:(j + 1) * 256], xt16[:],
                            yt[:, s:s + 1], None, OP.add)
                    nc.scalar.activation(tb[:], tp[:], AF.Tanh)
                    for j in range(8):
                        s = blk * 8 + j
                        sblk, sl = s // 128, s % 128
                        for h in range(2):
                            nc.tensor.matmul(
                                sc_ps[h][:, sblk * 256 + 2 * sl: sblk * 256 + 2 * sl + 2],
                                tb[:, j * 256 + h * 128: j * 256 + (h + 1) * 128],
                                va[:], start=True, stop=True)

                # ---- softmax pieces: e, Z, 1/Z broadcast ----
                eT = wp.tile([128, 1024], F32R, tag="eT")
                for h in range(2):
                    nc.scalar.activation(eT[:, h * 512:(h + 1) * 512], sc_ps[h][:],
                                         AF.Exp)
                if apply_mask:
                    # mask l where x_mask[b, l]: e *= {0,1}, per (l-half, b):
                    # partition = l, per-partition scalar = mask column
                    for h in range(2):
                        for b in range(2):
                            sl = eT[:, h * 512 + b: h * 512 + 512: 2]
                            nc.vector.tensor_scalar(
                                sl, sl, mk[:, h * 2 + b: h * 2 + b + 1], None,
                                OP.mult)
                z_ps = bank()
                for h in range(2):
                    nc.tensor.matmul(z_ps[0:1, :], onc[:], eT[:, h * 512:(h + 1) * 512],
                                     start=(h == 0), stop=(h == 1))
                rz = wp.tile([1, 512], F32R, tag="rz")
                nc.vector.reciprocal(rz[:], z_ps[0:1, :])
                rzb_ps = bank()
                for b in range(2):
                    nc.tensor.matmul(rzb_ps[:, b * 256:(b + 1) * 256], onr[:],
                                     rz[0:1, b:512:2], start=True, stop=True)
                rzb = wp.tile([128, 512], F32, tag="rzb")
                nc.vector.tensor_copy(rzb[:], rzb_ps[:])

                # ---- pools^T = (mem^T e) / Z ----
                poolsT = [wp.tile([128, 512], F32R, tag=f"poolsT{dh}",
                                  name=f"poolsT{li}_{dh}") for dh in range(2)]
                for dh in range(2):
                    pn_ps = bank()
                    for b in range(2):
                        for lh in range(2):
                            nc.tensor.matmul(
                                pn_ps[:, b * 256:(b + 1) * 256],
                                memr[:, lh * 512 + b * 256 + dh * 128:
                                     lh * 512 + b * 256 + (dh + 1) * 128],
                                eT[:, lh * 512 + b: lh * 512 + 512: 2],
                                start=(lh == 0), stop=(lh == 1))
                    nc.vector.scalar_tensor_tensor(
                        poolsT[dh][:], pn_ps[:], 1.0, rzb[:], OP.mult, OP.mult)

                # ---- SRU: U matmuls + gates + scan, both directions ----
                for dr in range(2):
                    u_ps = [bank() for _ in range(4)]
                    for c in range(4):
                        if c < 2:
                            rhs = (memT[:, c * 512:(c + 1) * 512] if li == 0
                                   else h0[c][:])
                        else:
                            rhs = poolsT[c - 2][:]
                        for jj in range(4):
                            w_off = (((li * 2 + dr) * 16) + c * 4 + jj) * 128
                            nc.tensor.matmul(u_ps[jj][:],
                                             wsru[:, w_off:w_off + 128], rhs,
                                             start=(c == 0), stop=(c == 3))
                    bcol = (li * 2 + dr) * 2
                    tf_ = sp.tile([128, 512], F32, tag="tf")
                    nc.scalar.activation(tf_[:], u_ps[1][:], AF.Tanh,
                                         bias=bsru[:, bcol:bcol + 1], scale=0.5)
                    f_ = sp.tile([128, 512], F32, tag="f")
                    nc.vector.tensor_scalar(f_[:], tf_[:], 0.5, 0.5, OP.mult, OP.add)
                    g_ = sp.tile([128, 512], F32, tag="g")
                    nc.vector.tensor_scalar(g_[:], tf_[:], -0.5, 0.5, OP.mult, OP.add)
                    bin_ = sp.tile([128, 512], F32, tag="bin")
                    nc.vector.tensor_tensor(bin_[:], g_[:], u_ps[0][:], OP.mult)
                    c_ = sp.tile([128, 512], F32, tag="c")
                    for b in range(2):
                        nc.vector.tensor_tensor_scan(
                            c_[:, b * 256:(b + 1) * 256],
                            f_[:, b * 256:(b + 1) * 256],
                            bin_[:, b * 256:(b + 1) * 256],
                            0.0, OP.mult, OP.add)
                    tc_ = sp.tile([128, 512], F32, tag="tc")
                    nc.scalar.activation(tc_[:], c_[:], AF.Tanh)
                    tr_ = sp.tile([128, 512], F32, tag="tr")
                    nc.scalar.activation(tr_[:], u_ps[2][:], AF.Tanh,
                                         bias=bsru[:, bcol + 1:bcol + 2], scale=0.5)
                    dd_ = sp.tile([128, 512], F32, tag="dd")
                    nc.vector.tensor_tensor(dd_[:], tc_[:], u_ps[3][:], OP.subtract)
                    rd2_ = sp.tile([128, 512], F32, tag="rd2")
                    nc.vector.scalar_tensor_tensor(rd2_[:], tr_[:], 1.0, dd_[:],
                                                   OP.add, OP.mult)
                    h_t = h0[dr] if li == 0 else h1[dr]
                    nc.vector.scalar_tensor_tensor(h_t[:], rd2_[:], 0.5, u_ps[3][:],
                                                   OP.mult, OP.add)

            for b in range(B2):
                for dh in range(2):
                    nc.sync.dma_start(outT_d[b, dh], h1[dh][:, b * 256:(b + 1) * 256])

    _split_excess_waits(nc)
    return nc


_CACHE = {}


def _get_nc(apply_mask: bool):
    if apply_mask not in _CACHE:
        _CACHE[apply_mask] = _build(apply_mask)
    return _CACHE[apply_mask]


def make_in_maps(x, x_mask, actions, w1, b1, w2, b2, v,
                 sru_w_f, sru_b_f, sru_w_b, sru_b_b):
    x = np.asarray(x, np.float32)
    x_mask = np.asarray(x_mask)
    actions = np.asarray(actions).astype(np.int64)
    w1 = np.asarray(w1, np.float32); b1 = np.asarray(b1, np.float32)
    w2 = np.asarray(w2, np.float32); b2 = np.asarray(b2, np.float32)
    v = np.asarray(v, np.float32)
    sru_w = [np.asarray(sru_w_f, np.float32), np.asarray(sru_w_b, np.float32)]
    sru_b = [np.asarray(sru_b_f, np.float32), np.asarray(sru_b_b, np.float32)]

    apply_mask = bool(x_mask.any())

    # wsru host layout: [hp, ((li*2+dir)*16 + c*4 + jj)*128 + m]
    wsru = np.empty((128, 8192), np.float32)
    for li in range(NL):
        for dr in range(2):
            w = sru_w[dr][li]  # (512, 512)
            blk = w.reshape(4, 128, 4, 128)  # [c, dp, jj, m]
            for c in range(4):
                for jj in range(4):
                    off = ((li * 2 + dr) * 16 + c * 4 + jj) * 128
                    wsru[:, off:off + 128] = blk[c, :, jj, :]
    bsru = np.empty((128, 8), np.float32)
    for li in range(NL):
        for dr in range(2):
            bb = sru_b[dr][li]  # (256,)
            bsru[:, (li * 2 + dr) * 2 + 0] = 0.5 * bb[0:128]
            bsru[:, (li * 2 + dr) * 2 + 1] = 0.5 * bb[128:256]

    in_maps = []
    for core in range(NCORES):
        gb = [B2 * core + b for b in range(B2)]
        xs = x[gb]  # (2, S, D)
        a = [int(actions[g]) for g in gb]
        # memT[dp, dh*512 + b*256 + l]
        arr = xs.transpose(2, 0, 1)  # (d, b, l)
        memT = np.empty((128, 1024), np.float32)
        for dh in range(2):
            memT[:, dh * 512:(dh + 1) * 512] = (
                arr[dh * 128:(dh + 1) * 128].reshape(128, 512))
        memr = np.empty((128, 1024), np.float32)
        arr2 = xs.transpose(1, 0, 2)  # (l, b, d)
        for lh in range(2):
            memr[:, lh * 512:(lh + 1) * 512] = (
                arr2[lh * 128:(lh + 1) * 128].reshape(128, 512))
        w1blk = np.zeros((128, 512), np.float32)
        w2blk = np.zeros((128, 512), np.float32)
        for b in range(2):
            for ci in range(2):
                cc = b * 2 + ci
                w1blk[:, cc * 128 + b * 64: cc * 128 + b * 64 + 64] = \
                    w1[a[b], ci * 128:(ci + 1) * 128, :]
                w2blk[:, cc * 128 + b * 64: cc * 128 + b * 64 + 64] = \
                    w2[a[b], ci * 128:(ci + 1) * 128, :]
        vablk = np.zeros((128, 2), np.float32)
        ybias = np.zeros((128, 1), np.float32)
        for b in range(2):
            vablk[b * 64:(b + 1) * 64, b] = v[a[b]]
            ybias[b * 64:(b + 1) * 64, 0] = b1[a[b]] + b2[a[b]]
        m = {
            "memT": memT, "memr": memr, "w1blk": w1blk, "w2blk": w2blk,
            "vablk": vablk.astype(BF16_NP), "ybias": ybias,
            "wsru": wsru, "bsru": bsru,
            "onescol": np.ones((128, 1), np.float32),
            "onesrow": np.ones((1, 128), np.float32),
        }
        if apply_mask:
            mk = np.empty((128, 4), np.float32)
            for lh in range(2):
                for b in range(2):
                    mk[:, lh * 2 + b] = np.where(
                        x_mask[gb[b], lh * 128:(lh + 1) * 128], 0.0, 1.0)
            m["maskmul"] = mk
        in_maps.append(m)
    return in_maps, apply_mask


def assemble_output(results):
    y = np.empty((B, S, D), np.float32)
    for core in range(NCORES):
        outT = results[core]["outT"]  # (2, 2, 128, 256)
        y[B2 * core: B2 * (core + 1)] = (
            outT.transpose(0, 3, 1, 2).reshape(B2, S, D))
    return y


def kernel(**inputs) -> np.ndarray:
    in_maps, apply_mask = make_in_maps(**inputs)
    nc = _get_nc(apply_mask)
    res = run_bass_kernel_spmd(nc, in_maps, list(range(NCORES)))
    return assemble_output(res.results)
